# revision 1
# baseline (speedup 1.0000x reference)
"""Trainium2 Bass kernel for nn_CosSimSpatTempConvNet.

Math (reference):
  merged[f,c,k] = conv_w[f,k] * spat_w[f,c]                  (rank-1 kernel)
  conved[b,f,t] = sum_{c,k} merged[f,c,k] * x[b,c,t+k]       (valid conv, Tout=T-K+1)
  norm_w[f]    = ||conv_w[f]|| * ||spat_w[f]||
  norm_in[b,t] = sqrt(sum_{c,k} x[b,c,t+k]^2)
  cos[b,f,t]   = conved * 64 / (norm_w[f] * norm_in[b,t])
  out[b]       = sum_f (mean_t |cos[b,f,t]| * weight[f] + bias[f])

Device strategy (8 cores, data-parallel over batch, 8 b per core):
  * Full conv as TensorE matmuls with contraction dim 128 = (c, k2):
    x2 tile holds x[b] on partitions 0-63 and x[b] shifted by one time-step
    on partitions 64-127, so 32 PSUM-accumulated matmuls (one per k-pair)
    with moving-operand slices x2[:, t0+2*kp : ...] produce conved[:, t0:t0+512].
    Stationaries W2[kp][(c,k2), f] = conv_w[f, 2kp+k2]*spat_w[f,c] are
    host-precomputed (tiny) and passed as an input.
  * fp32 data, matmuls issued as float32r (full-rate for N>=256).
  * norm_in: x^2 on ScalarE, sum over c via ones-stationary matmul,
    sliding-window-64 sum via 6 doubling shift-adds on VectorE,
    reciprocal (VectorE) + sqrt (ScalarE).
  * epilogue per (b, t-tile): broadcast nrecip row across 128 partitions with
    a K=1 matmul, then one fused VectorE scalar_tensor_tensor:
    (|conved| via abs_max with 0) * nrecip, accumulated over t (accum_out).
  * finish: S[f,b] sums -> one matmul with stationary g[f] = 64*weight/(4033*norm_w)
    contracting over f, plus host-computed sum(bias).
"""

import contextlib
import ctypes
import sys
import types

import numpy as np

import concourse.bass as bass
import concourse.mybir as mybir
import concourse.tile as tile
from concourse.bass_utils import run_bass_kernel_spmd
from concourse.vector_clock import ScopedClock

F32 = mybir.dt.float32
F32R = mybir.dt.float32r

B, CIN, T = 64, 64, 4096
F, K = 128, 64
TOUT = T - K + 1          # 4033
NCORES = 8
BLOC = B // NCORES        # 8 batches per core
KP = K // 2               # 32 packed k-pairs
TS = 512                  # moving-operand tile (one fp32 PSUM bank)
NTILES = (TOUT + TS - 1) // TS      # 8 (last tile 449)
SCALE = 64.0              # sqrt(CIN*K)

AF = mybir.ActivationFunctionType
ALU = mybir.AluOpType


# ---------------------------------------------------------------------------
# Container fixups: walrus here rejects >1 sem-wait on a Drain; TileContext's
# tail drain carries one wait per logical processor.  Chunk into single-wait
# drains.  Also recreate the (absent) antenv.axon_hooks NTFF profile hook so
# trace=True works when a test harness wants timings.
# ---------------------------------------------------------------------------

def _patched_drain_and_barrier(self, tick_clock, wait_clock):
    nc = self.nc
    drain_inst = nc.sync.drain()
    wait_clock.add_sem_waits(
        drain_inst.ins, ScopedClock({None: tick_clock.global_clock})
    )
    si = drain_inst.ins.sync_info
    waits = list(si.on_wait or []) if si else []
    if len(waits) > 1:
        si.on_wait = waits[:1]
        for w in waits[1:]:
            d2 = nc.sync.drain()
            si2 = d2.ins.sync_info
            if si2 is None:
                d2.ins.sync_info = mybir.SyncInfo(on_wait=[w], on_update=[])
            else:
                si2.on_wait = [w]
    nc.all_engine_barrier()
    assert self.sems is not None
    popped = nc._tile_sem_poison_stack.pop()
    assert popped is self._sem_poison
    nc.clear_and_free_semaphores(list(self.sems.allocated().values()))
    nc.all_engine_barrier()


def _install_ntff_hook():
    if "antenv.axon_hooks" in sys.modules:
        return
    try:
        lib = ctypes.CDLL("/opt/axon/libaxon_pjrt.so")
    except OSError:
        return
    if not hasattr(lib, "axon_start_nrt_profile"):
        return
    lib.axon_start_nrt_profile.argtypes = [
        ctypes.POINTER(ctypes.c_int64),
        ctypes.c_size_t,
    ]
    lib.axon_start_nrt_profile.restype = ctypes.c_int64
    lib.axon_stop_nrt_profile.argtypes = [ctypes.c_char_p]
    lib.axon_stop_nrt_profile.restype = ctypes.c_int64

    @contextlib.contextmanager
    def _hook(output_dir, device_ids):
        import jax

        jax.devices()
        if device_ids:
            ids = (ctypes.c_int64 * len(device_ids))(*device_ids)
            rc = lib.axon_start_nrt_profile(ids, len(device_ids))
        else:
            rc = lib.axon_start_nrt_profile(None, 0)
        if rc != 0:
            raise RuntimeError(f"axon_start_nrt_profile rc={rc}")
        try:
            yield
        finally:
            n = lib.axon_stop_nrt_profile(str(output_dir).encode())
            print(f"profile: {n} ntff file(s) in {output_dir}", file=sys.stderr)

    mod = types.ModuleType("antenv.axon_hooks")
    mod.get_axon_ntff_profile_hook = lambda: _hook
    mod.set_axon_ntff_profile_hook = lambda h: None
    import antenv

    antenv.axon_hooks = mod
    sys.modules["antenv.axon_hooks"] = mod


_ORIG_COMMIT = tile.TileContext._commit_instruction


def _commit_split_waits(self, inst, lazy_reg_writes=True):
    """walrus here allows only one sem-wait per instruction; move extras
    onto same-engine NOPs committed immediately before the instruction."""
    si = getattr(inst, "sync_info", None)
    if (
        si is not None
        and si.on_wait
        and len(si.on_wait) > 1
        and inst.engine != mybir.EngineType.Unassigned
    ):
        waits = list(si.on_wait)
        si.on_wait = waits[:1]
        for i, w in enumerate(waits[1:]):
            nop = mybir.InstNoOp(
                name=f"{inst.name}-wsplit{i}", ins=[], outs=[]
            )
            nop.engine = inst.engine
            nop.sync_info = mybir.SyncInfo(on_wait=[w], on_update=[])
            _ORIG_COMMIT(self, nop, lazy_reg_writes=False)
    return _ORIG_COMMIT(self, inst, lazy_reg_writes)


def install_fixups():
    tile.TileContext._drain_and_barrier = _patched_drain_and_barrier
    tile.TileContext._commit_instruction = _commit_split_waits
    _install_ntff_hook()


# ---------------------------------------------------------------------------
# Device program (identical on all 8 cores; inputs differ per core)
# ---------------------------------------------------------------------------

def build_program() -> bass.Bass:
    install_fixups()
    nc = bass.Bass()

    xs_in = nc.dram_tensor("xs", [BLOC, CIN, T], F32, kind="ExternalInput")
    w2_in = nc.dram_tensor("w2", [128, KP, F], F32, kind="ExternalInput")
    g_in = nc.dram_tensor("g", [F, 1], F32, kind="ExternalInput")
    bsum_in = nc.dram_tensor("bsum", [1, 1], F32, kind="ExternalInput")
    ones64_in = nc.dram_tensor("ones64", [64, 1], F32, kind="ExternalInput")
    ones1_in = nc.dram_tensor("ones1", [1, 128], F32, kind="ExternalInput")
    zcol_in = nc.dram_tensor("zcol", [64, 1], F32, kind="ExternalInput")
    out_d = nc.dram_tensor("out", [1, BLOC], F32, kind="ExternalOutput")

    PS = bass.MemorySpace.PSUM

    with tile.TileContext(nc) as tc:
        with (
            tc.tile_pool(name="const", bufs=1) as constp,
            tc.tile_pool(name="xp", bufs=2) as xp,
            tc.tile_pool(name="sqp", bufs=2) as sqp,
            tc.tile_pool(name="bigp", bufs=3) as bigp,
            tc.tile_pool(name="rowp", bufs=3) as rowp,
            tc.tile_pool(name="scrp", bufs=2) as scrp,
            tc.tile_pool(name="accp", bufs=2) as accp,
            tc.tile_pool(name="pconv", bufs=3, space=PS) as pconv,
            tc.tile_pool(name="pbc", bufs=2, space=PS) as pbc,
            tc.tile_pool(name="psq", bufs=2, space=PS) as psq,
            tc.tile_pool(name="pfin", bufs=1, space=PS) as pfin,
        ):
            # constants.  Tiles feeding float32r matmuls are declared f32r so
            # every producer instruction emits "rounded" f32r (BIR verifier
            # requirement); DRAM stays f32 and DMAs bitcast the source.
            w2sb = constp.tile([128, KP, F], F32R)
            nc.sync.dma_start(w2sb[:], w2_in[:].bitcast(F32R))
            gsb = constp.tile([F, 1], F32)
            nc.sync.dma_start(gsb[:], g_in[:])
            bsumsb = constp.tile([1, 1], F32)
            nc.sync.dma_start(bsumsb[:], bsum_in[:])
            ones64 = constp.tile([64, 1], F32R)
            nc.sync.dma_start(ones64[:], ones64_in[:].bitcast(F32R))
            ones1 = constp.tile([1, 128], F32R)
            nc.sync.dma_start(ones1[:], ones1_in[:].bitcast(F32R))
            zcol = constp.tile([64, 1], F32R)
            nc.sync.dma_start(zcol[:], zcol_in[:].bitcast(F32R))

            sq_all = constp.tile([BLOC, T], F32)   # sum_c x^2, one row per b
            S = constp.tile([F, BLOC], F32)        # per-(f,b) |cos| sums

            # ---- phase 1: input norms ------------------------------------
            for b in range(BLOC):
                xt = xp.tile([128, T], F32R, tag="xt")
                nc.sync.dma_start(xt[0:64, :], xs_in[b].bitcast(F32R))
                xsq = sqp.tile([64, T], F32R, tag="xsq")
                nc.scalar.activation(xsq[:], xt[0:64, :], AF.Square)
                for ts in range(T // TS):
                    pq = psq.tile([1, TS], F32)
                    nc.tensor.matmul(
                        pq[:],
                        ones64[:],
                        xsq[:, ts * TS:(ts + 1) * TS],
                    )
                    sqrow = rowp.tile([1, TS], F32, tag="sqrow")
                    nc.scalar.copy(sqrow[:], pq[:])
                    # cross-partition row placement -> DMA, not DVE
                    nc.sync.dma_start(
                        sq_all[b:b + 1, ts * TS:(ts + 1) * TS], sqrow[:]
                    )

            # sliding-window-64 sum via doubling shifts, then 1/sqrt
            cur = sq_all
            width = T
            for sh in (1, 2, 4, 8, 16, 32):
                width -= sh
                nxt = bigp.tile([BLOC, T], F32, tag="slide")
                nc.vector.tensor_tensor(
                    nxt[:, 0:width], cur[:, 0:width], cur[:, sh:sh + width],
                    op=ALU.add,
                )
                cur = nxt
            assert width == TOUT
            recip_t = bigp.tile([BLOC, T], F32, tag="slide")
            nc.vector.reciprocal(recip_t[:, 0:TOUT], cur[:, 0:TOUT])
            nrec8 = constp.tile([BLOC, T], F32R)
            nc.scalar.activation(nrec8[:, 0:TOUT], recip_t[:, 0:TOUT], AF.Sqrt)

            # ---- phase 2: conv + cosine epilogue -------------------------
            for b in range(BLOC):
                x2 = xp.tile([128, T], F32R, tag="xt")
                nc.sync.dma_start(x2[0:64, :], xs_in[b].bitcast(F32R))
                nc.sync.dma_start(x2[64:128, 0:T - 1], x2[0:64, 1:T])
                nc.sync.dma_start(x2[64:128, T - 1:T], zcol[:])
                acc = accp.tile([F, NTILES], F32)
                for ts in range(NTILES):
                    t0 = ts * TS
                    nt = min(TS, TOUT - t0)
                    nt_mm = nt + (nt & 1)   # f32r moving free size must be even
                    pc = pconv.tile([F, TS], F32)
                    for kp in range(KP):
                        nc.tensor.matmul(
                            pc[:, 0:nt_mm],
                            w2sb[:, kp, :],
                            x2[:, t0 + 2 * kp: t0 + 2 * kp + nt_mm],
                            start=(kp == 0),
                            stop=(kp == KP - 1),
                        )
                    nrow = rowp.tile([1, TS], F32R)
                    nc.sync.dma_start(nrow[0:1, 0:nt], nrec8[b:b + 1, t0:t0 + nt])
                    if nt_mm != nt:
                        nc.sync.dma_start(nrow[0:1, nt:nt_mm], zcol[0:1, :].bitcast(F32R))
                    pb = pbc.tile([128, TS], F32)
                    nc.tensor.matmul(
                        pb[:, 0:nt_mm],
                        ones1[:],
                        nrow[0:1, 0:nt_mm],
                    )
                    # DVE may read only one PSUM operand: take |conv| on
                    # ScalarE (PSUM->SBUF), then fuse multiply+row-sum on DVE.
                    scr = scrp.tile([F, TS], F32)
                    nc.scalar.activation(scr[:, 0:nt], pc[:, 0:nt], AF.Abs)
                    nc.vector.scalar_tensor_tensor(
                        scr[:, 0:nt],
                        scr[:, 0:nt],
                        1.0,
                        pb[:, 0:nt],
                        op0=ALU.mult,
                        op1=ALU.mult,
                        accum_out=acc[:, ts:ts + 1],
                    )
                nc.vector.reduce_sum(
                    S[:, b:b + 1], acc[:], axis=mybir.AxisListType.X
                )

            # ---- finish: out[b] = sum_f g[f]*S[f,b] + sum_f bias[f] ------
            pf = pfin.tile([1, BLOC], F32)
            nc.tensor.matmul(pf[:], gsb[:], S[:])
            out_sb = constp.tile([1, BLOC], F32)
            nc.scalar.add(out_sb[:], pf[:], bsumsb[0:1, 0:1])
            nc.sync.dma_start(out_d[:], out_sb[:])

    return nc


_PROGRAM: bass.Bass | None = None


def _get_program() -> bass.Bass:
    global _PROGRAM
    if _PROGRAM is None:
        _PROGRAM = build_program()
    return _PROGRAM


# ---------------------------------------------------------------------------
# Host entry point
# ---------------------------------------------------------------------------

def host_params(conv_weights, spat_weights, weight, bias):
    """Tiny host-side precomputation of stationaries and scalars."""
    conv = np.asarray(conv_weights, dtype=np.float64)
    spat = np.asarray(spat_weights, dtype=np.float64)
    w = np.asarray(weight, dtype=np.float64)
    bb = np.asarray(bias, dtype=np.float64)

    # W2[k, c, f] = conv[f, k] * spat[f, c]; pack partition index (k2, c)
    W = np.einsum("fk,fc->kcf", conv, spat)            # [K, C, F]
    V = W.reshape(KP, 2, CIN, F).reshape(KP, 128, F)   # [(kp), (k2,c), F]
    w2 = np.ascontiguousarray(V.transpose(1, 0, 2)).astype(np.float32)

    norm_w = np.sqrt((spat * spat).sum(1) * (conv * conv).sum(1))  # [F]
    g = (SCALE / (TOUT * norm_w) * w).astype(np.float32).reshape(F, 1)
    bsum = np.array([[bb.sum()]], dtype=np.float32)
    return w2, g, bsum


def kernel(x, conv_weights, spat_weights, weight, bias):
    x = np.ascontiguousarray(np.asarray(x, dtype=np.float32))
    w2, g, bsum = host_params(conv_weights, spat_weights, weight, bias)

    nc = _get_program()
    in_maps = []
    for c in range(NCORES):
        in_maps.append(
            {
                "xs": np.ascontiguousarray(x[c * BLOC:(c + 1) * BLOC]),
                "w2": w2,
                "g": g,
                "bsum": bsum,
                "ones64": np.ones((64, 1), np.float32),
                "ones1": np.ones((1, 128), np.float32),
                "zcol": np.zeros((64, 1), np.float32),
            }
        )
    res = run_bass_kernel_spmd(nc, in_maps, core_ids=list(range(NCORES)))
    out = np.concatenate(
        [res.results[c]["out"].reshape(BLOC) for c in range(NCORES)]
    )
    return out.astype(np.float32)



# revision 2
# speedup vs baseline: 1.9428x; 1.9428x over previous
"""Trainium2 Bass kernel for nn_CosSimSpatTempConvNet.

Math (reference):
  merged[f,c,k] = conv_w[f,k] * spat_w[f,c]                  (rank-1 kernel)
  conved[b,f,t] = sum_{c,k} merged[f,c,k] * x[b,c,t+k]       (valid conv, Tout=T-K+1)
  norm_w[f]    = ||conv_w[f]|| * ||spat_w[f]||
  norm_in[b,t] = sqrt(sum_{c,k} x[b,c,t+k]^2)
  cos[b,f,t]   = conved * 64 / (norm_w[f] * norm_in[b,t])
  out[b]       = sum_f (mean_t |cos[b,f,t]| * weight[f] + bias[f])

Device strategy (8 cores, data-parallel over batch, 8 b per core):
  * Conv as fp8e4 DoubleRow TensorE matmuls: contraction 256 per
    instruction = 128 partitions (k2 in {0,1} x c) x 2 k-planes
    (j in {0,1}; k = 2*kq + k2 + 32*j).  16 PSUM-accumulated matmuls per
    512-wide time tile (vs 32 for fp32r), 0.5 cycles/output-row.
    Stationaries wdr[(k2,c), kq, j, f] = conv_w[f,2kq+k2+32j]*spat_w[f,c]
    host-precomputed, scaled by SW=128, clipped to +-240 (TRN e4m3).
  * Moving operand xf8[(k2,c), j, t] = 16*x[c, t+k2+32j] in fp8:
    plane builds via ScalarE/DVE casts + one SBUF shift-DMA.
  * norm_in: x^2 (ScalarE), sum over c via ones-stationary matmul,
    then per-batch chunked layout [16 chunks, 256+64 halo] so the
    sliding-window-64 doubling shifts + reciprocal run on short rows.
  * epilogue per (b, t-tile): broadcast 1/norm row across 128 partitions
    with a K=1 matmul; |conved| via ScalarE Abs; fused multiply+row-sum
    on DVE (accum_out).
  * finish: S[f,b] sums -> one matmul with stationary
    g[f] = 64*weight/(4033*norm_w*SW*SX) contracting over f, + sum(bias).
"""

import contextlib
import ctypes
import sys
import types

import numpy as np

import concourse.bass as bass
import concourse.mybir as mybir
import concourse.tile as tile
from concourse.bass_utils import run_bass_kernel_spmd
from concourse.vector_clock import ScopedClock

F32 = mybir.dt.float32
F32R = mybir.dt.float32r
F8 = mybir.dt.float8e4

B, CIN, T = 64, 64, 4096
F, K = 128, 64
TOUT = T - K + 1          # 4033
NCORES = 8
BLOC = B // NCORES        # 8 batches per core
KQ = 16                   # DoubleRow quad groups: k = 2*kq + k2 + 32*j
TS = 512                  # moving-operand tile (one fp32 PSUM bank)
NTILES = (TOUT + TS - 1) // TS      # 8 (last tile 449)
SCALE = 64.0              # sqrt(CIN*K)
SW = 128.0                # fp8 weight scale
SX = 16.0                 # fp8 x scale
NCH = 16                  # norm chunks per batch
CW = 256                  # chunk output width
CPAD = 320                # chunk width incl. 64-halo

AF = mybir.ActivationFunctionType
ALU = mybir.AluOpType
DR = mybir.MatmulPerfMode.DoubleRow


# ---------------------------------------------------------------------------
# Container fixups: walrus here rejects >1 sem-wait on a Drain; TileContext's
# tail drain carries one wait per logical processor.  Chunk into single-wait
# drains.  Also recreate the (absent) antenv.axon_hooks NTFF profile hook so
# trace=True works when a test harness wants timings.
# ---------------------------------------------------------------------------

def _patched_drain_and_barrier(self, tick_clock, wait_clock):
    nc = self.nc
    drain_inst = nc.sync.drain()
    wait_clock.add_sem_waits(
        drain_inst.ins, ScopedClock({None: tick_clock.global_clock})
    )
    si = drain_inst.ins.sync_info
    waits = list(si.on_wait or []) if si else []
    if len(waits) > 1:
        si.on_wait = waits[:1]
        for w in waits[1:]:
            d2 = nc.sync.drain()
            si2 = d2.ins.sync_info
            if si2 is None:
                d2.ins.sync_info = mybir.SyncInfo(on_wait=[w], on_update=[])
            else:
                si2.on_wait = [w]
    nc.all_engine_barrier()
    assert self.sems is not None
    popped = nc._tile_sem_poison_stack.pop()
    assert popped is self._sem_poison
    nc.clear_and_free_semaphores(list(self.sems.allocated().values()))
    nc.all_engine_barrier()


def _install_ntff_hook():
    if "antenv.axon_hooks" in sys.modules:
        return
    try:
        lib = ctypes.CDLL("/opt/axon/libaxon_pjrt.so")
    except OSError:
        return
    if not hasattr(lib, "axon_start_nrt_profile"):
        return
    lib.axon_start_nrt_profile.argtypes = [
        ctypes.POINTER(ctypes.c_int64),
        ctypes.c_size_t,
    ]
    lib.axon_start_nrt_profile.restype = ctypes.c_int64
    lib.axon_stop_nrt_profile.argtypes = [ctypes.c_char_p]
    lib.axon_stop_nrt_profile.restype = ctypes.c_int64

    @contextlib.contextmanager
    def _hook(output_dir, device_ids):
        import jax

        jax.devices()
        if device_ids:
            ids = (ctypes.c_int64 * len(device_ids))(*device_ids)
            rc = lib.axon_start_nrt_profile(ids, len(device_ids))
        else:
            rc = lib.axon_start_nrt_profile(None, 0)
        if rc != 0:
            raise RuntimeError(f"axon_start_nrt_profile rc={rc}")
        try:
            yield
        finally:
            n = lib.axon_stop_nrt_profile(str(output_dir).encode())
            print(f"profile: {n} ntff file(s) in {output_dir}", file=sys.stderr)

    mod = types.ModuleType("antenv.axon_hooks")
    mod.get_axon_ntff_profile_hook = lambda: _hook
    mod.set_axon_ntff_profile_hook = lambda h: None
    import antenv

    antenv.axon_hooks = mod
    sys.modules["antenv.axon_hooks"] = mod


_ORIG_COMMIT = tile.TileContext._commit_instruction


def _commit_split_waits(self, inst, lazy_reg_writes=True):
    """walrus here allows only one sem-wait per instruction; move extras
    onto same-engine NOPs committed immediately before the instruction."""
    si = getattr(inst, "sync_info", None)
    if (
        si is not None
        and si.on_wait
        and len(si.on_wait) > 1
        and inst.engine != mybir.EngineType.Unassigned
    ):
        waits = list(si.on_wait)
        si.on_wait = waits[:1]
        for i, w in enumerate(waits[1:]):
            nop = mybir.InstNoOp(
                name=f"{inst.name}-wsplit{i}", ins=[], outs=[]
            )
            nop.engine = inst.engine
            nop.sync_info = mybir.SyncInfo(on_wait=[w], on_update=[])
            _ORIG_COMMIT(self, nop, lazy_reg_writes=False)
    return _ORIG_COMMIT(self, inst, lazy_reg_writes)


def install_fixups():
    tile.TileContext._drain_and_barrier = _patched_drain_and_barrier
    tile.TileContext._commit_instruction = _commit_split_waits
    _install_ntff_hook()


# ---------------------------------------------------------------------------
# Device program (identical on all 8 cores; inputs differ per core)
# ---------------------------------------------------------------------------

def build_program() -> bass.Bass:
    install_fixups()
    nc = bass.Bass()

    xs_in = nc.dram_tensor("xs", [BLOC, CIN, T], F32, kind="ExternalInput")
    wdr_in = nc.dram_tensor("wdr", [128, KQ, 2, F], F8, kind="ExternalInput")
    g_in = nc.dram_tensor("g", [F, 1], F32, kind="ExternalInput")
    bsum_in = nc.dram_tensor("bsum", [1, 1], F32, kind="ExternalInput")
    ones64_in = nc.dram_tensor("ones64", [64, 1], F32, kind="ExternalInput")
    ones1_in = nc.dram_tensor("ones1", [1, 128], F32, kind="ExternalInput")
    out_d = nc.dram_tensor("out", [1, BLOC], F32, kind="ExternalOutput")

    PS = bass.MemorySpace.PSUM

    with tile.TileContext(nc) as tc:
        with (
            tc.tile_pool(name="const", bufs=1) as constp,
            tc.tile_pool(name="xtp", bufs=2) as xtp,
            tc.tile_pool(name="sqp", bufs=2) as sqp,
            tc.tile_pool(name="xf8p", bufs=2) as xf8p,
            tc.tile_pool(name="rowp", bufs=2) as rowp,
            tc.tile_pool(name="slidep", bufs=3) as slidep,
            tc.tile_pool(name="rcp", bufs=2) as rcp,
            tc.tile_pool(name="nrowp", bufs=3) as nrowp,
            tc.tile_pool(name="scrp", bufs=2) as scrp,
            tc.tile_pool(name="accp", bufs=2) as accp,
            tc.tile_pool(name="pconv", bufs=3, space=PS) as pconv,
            tc.tile_pool(name="pbc", bufs=2, space=PS) as pbc,
            tc.tile_pool(name="psq", bufs=2, space=PS) as psq,
            tc.tile_pool(name="pfin", bufs=1, space=PS) as pfin,
        ):
            # constants
            wdr = constp.tile([128, KQ, 2, F], F8)
            nc.sync.dma_start(wdr[:], wdr_in[:])
            gsb = constp.tile([F, 1], F32)
            nc.sync.dma_start(gsb[:], g_in[:])
            bsumsb = constp.tile([1, 1], F32)
            nc.sync.dma_start(bsumsb[:], bsum_in[:])
            ones64 = constp.tile([64, 1], F32R)
            nc.sync.dma_start(ones64[:], ones64_in[:].bitcast(F32R))
            ones1 = constp.tile([1, 128], F32R)
            nc.sync.dma_start(ones1[:], ones1_in[:].bitcast(F32R))

            S = constp.tile([F, BLOC], F32)        # per-(f,b) |cos| sums

            for b in range(BLOC):
                # ---- load + squares -------------------------------------
                xt = xtp.tile([64, T], F32, tag="xt")
                nc.sync.dma_start(xt[:], xs_in[b])
                xsq = sqp.tile([64, T], F32R, tag="xsq")
                nc.scalar.activation(xsq[:], xt[:], AF.Square)

                # sum over c via ones-stationary matmuls; write rows into
                # the chunk-major staging row [1, 17, 256] (pad chunk = 1.0)
                srow = rowp.tile([1, NCH + 1, CW], F32, tag="srow")
                nc.vector.memset(srow[0:1, NCH, :], 1.0)
                for ts in range(T // TS):
                    pq = psq.tile([1, TS], F32)
                    nc.tensor.matmul(
                        pq[:],
                        ones64[:],
                        xsq[:, ts * TS:(ts + 1) * TS],
                    )
                    nc.scalar.copy(srow[0:1, 2 * ts, :], pq[0:1, 0:CW])
                    nc.scalar.copy(srow[0:1, 2 * ts + 1, :], pq[0:1, CW:TS])

                # chunked [16, 320] = [body 256 | halo 64] via reshape DMAs
                sqc = slidep.tile([NCH, CPAD], F32, tag="slide")
                nc.sync.dma_start(sqc[:, 0:CW], srow[0:1, 0:NCH, :])
                nc.sync.dma_start(sqc[:, CW:CPAD], srow[0:1, 1:NCH + 1, 0:64])

                # sliding-window-64 sum via doubling shifts, then 1/sqrt
                cur = sqc
                width = CPAD
                for sh in (1, 2, 4, 8, 16, 32):
                    width -= sh
                    nxt = slidep.tile([NCH, CPAD], F32, tag="slide")
                    nc.vector.tensor_tensor(
                        nxt[:, 0:width], cur[:, 0:width], cur[:, sh:sh + width],
                        op=ALU.add,
                    )
                    cur = nxt
                assert width == CW + 1
                rec = slidep.tile([NCH, CPAD], F32, tag="slide")
                nc.vector.reciprocal(rec[:, 0:CW], cur[:, 0:CW])
                rc = rcp.tile([NCH, CW], F32R, tag="rc")
                nc.scalar.activation(rc[:], rec[:, 0:CW], AF.Sqrt)

                # ---- fp8 moving operand ---------------------------------
                # xf8[(k2,c), j, t] = SX * x[c, t + k2 + 32j]
                xf8 = xf8p.tile([128, 2, T], F8, tag="xf8")
                nc.vector.memset(xf8[:, 0:2, T - 64:T], 0.0)
                nc.scalar.activation(
                    xf8[0:64, 0, 0:T], xt[:], AF.Copy, scale=SX
                )
                nc.vector.tensor_scalar_mul(
                    xf8[0:64, 1, 0:T - 32], xt[0:64, 32:T], SX
                )
                nc.sync.dma_start(
                    xf8[64:128, 0:2, 0:T - 1], xf8[0:64, 0:2, 1:T]
                )

                # ---- conv + cosine epilogue -----------------------------
                acc = accp.tile([F, NTILES], F32)
                for ts in range(NTILES):
                    t0 = ts * TS
                    nt = min(TS, TOUT - t0)
                    nt_mm = nt + (nt & 1)   # even moving free size
                    pc = pconv.tile([F, TS], F32)
                    for kq in range(KQ):
                        nc.tensor.matmul(
                            pc[:, 0:nt_mm],
                            wdr[:, kq, 0:2, :],
                            xf8[:, 0:2, t0 + 2 * kq: t0 + 2 * kq + nt_mm],
                            start=(kq == 0),
                            stop=(kq == KQ - 1),
                            perf_mode=DR,
                        )
                    nrow = nrowp.tile([1, TS], F32R)
                    nc.sync.dma_start(
                        nrow[0:1, 0:TS], rc[2 * ts:2 * ts + 2, 0:CW]
                    )
                    pb = pbc.tile([128, TS], F32)
                    nc.tensor.matmul(pb[:], ones1[:], nrow[:])
                    # DVE may read only one PSUM operand: take |conv| on
                    # ScalarE (PSUM->SBUF), then fuse multiply+row-sum on DVE.
                    scr = scrp.tile([F, TS], F32)
                    nc.scalar.activation(scr[:, 0:nt], pc[:, 0:nt], AF.Abs)
                    nc.vector.scalar_tensor_tensor(
                        scr[:, 0:nt],
                        scr[:, 0:nt],
                        1.0,
                        pb[:, 0:nt],
                        op0=ALU.mult,
                        op1=ALU.mult,
                        accum_out=acc[:, ts:ts + 1],
                    )
                nc.vector.reduce_sum(
                    S[:, b:b + 1], acc[:], axis=mybir.AxisListType.X
                )

            # ---- finish: out[b] = sum_f g[f]*S[f,b] + sum_f bias[f] ------
            pf = pfin.tile([1, BLOC], F32)
            nc.tensor.matmul(pf[:], gsb[:], S[:])
            out_sb = constp.tile([1, BLOC], F32)
            nc.scalar.add(out_sb[:], pf[:], bsumsb[0:1, 0:1])
            nc.sync.dma_start(out_d[:], out_sb[:])

    return nc


_PROGRAM: bass.Bass | None = None


def _get_program() -> bass.Bass:
    global _PROGRAM
    if _PROGRAM is None:
        _PROGRAM = build_program()
    return _PROGRAM


# ---------------------------------------------------------------------------
# Host entry point
# ---------------------------------------------------------------------------

def host_params(conv_weights, spat_weights, weight, bias):
    """Tiny host-side precomputation of stationaries and scalars."""
    conv = np.asarray(conv_weights, dtype=np.float64)
    spat = np.asarray(spat_weights, dtype=np.float64)
    w = np.asarray(weight, dtype=np.float64)
    bb = np.asarray(bias, dtype=np.float64)

    # prod[k, c, f] = conv[f, k] * spat[f, c]; DoubleRow packing
    # k = 32*j + 2*kq + k2  ->  wdr[(k2,c), kq, j, f]
    prod = np.einsum("fk,fc->kcf", conv, spat) * SW    # [K, C, F]
    P5 = prod.reshape(2, KQ, 2, CIN, F)                # [j, kq, k2, c, f]
    wdr = np.ascontiguousarray(P5.transpose(2, 3, 1, 0, 4)).reshape(
        128, KQ, 2, F
    )
    wdr = np.clip(wdr, -240.0, 240.0)
    f8np = mybir.dt.np(F8)
    wdr = wdr.astype(np.float32).astype(f8np)

    norm_w = np.sqrt((spat * spat).sum(1) * (conv * conv).sum(1))  # [F]
    g = (SCALE / (TOUT * norm_w * SW * SX) * w).astype(np.float32).reshape(F, 1)
    bsum = np.array([[bb.sum()]], dtype=np.float32)
    return wdr, g, bsum


def make_in_maps(x, conv_weights, spat_weights, weight, bias):
    x = np.ascontiguousarray(np.asarray(x, dtype=np.float32))
    wdr, g, bsum = host_params(conv_weights, spat_weights, weight, bias)
    in_maps = []
    for c in range(NCORES):
        in_maps.append(
            {
                "xs": np.ascontiguousarray(x[c * BLOC:(c + 1) * BLOC]),
                "wdr": wdr,
                "g": g,
                "bsum": bsum,
                "ones64": np.ones((64, 1), np.float32),
                "ones1": np.ones((1, 128), np.float32),
            }
        )
    return in_maps


def kernel(x, conv_weights, spat_weights, weight, bias):
    in_maps = make_in_maps(x, conv_weights, spat_weights, weight, bias)
    nc = _get_program()
    res = run_bass_kernel_spmd(nc, in_maps, core_ids=list(range(NCORES)))
    out = np.concatenate(
        [res.results[c]["out"].reshape(BLOC) for c in range(NCORES)]
    )
    return out.astype(np.float32)


# revision 13
# speedup vs baseline: 1.9937x; 1.0262x over previous
"""Trainium2 Bass kernel for nn_CosSimSpatTempConvNet.

Math (reference):
  merged[f,c,k] = conv_w[f,k] * spat_w[f,c]                  (rank-1 kernel)
  conved[b,f,t] = sum_{c,k} merged[f,c,k] * x[b,c,t+k]       (valid conv, Tout=T-K+1)
  norm_w[f]    = ||conv_w[f]|| * ||spat_w[f]||
  norm_in[b,t] = sqrt(sum_{c,k} x[b,c,t+k]^2)
  cos[b,f,t]   = conved * 64 / (norm_w[f] * norm_in[b,t])
  out[b]       = sum_f (mean_t |cos[b,f,t]| * weight[f] + bias[f])

Device strategy (8 cores, data-parallel over batch, 8 b per core):
  * Conv as fp8e4 DoubleRow TensorE matmuls: contraction 256 per
    instruction = 128 partitions (k2 in {0,1} x c) x 2 k-planes
    (j in {0,1}; k = 2*kq + k2 + 32*j).  16 PSUM-accumulated matmuls per
    512-wide time tile (vs 32 for fp32r), 0.5 cycles/output-row.
    Stationaries wdr[(k2,c), kq, j, f] = conv_w[f,2kq+k2+32j]*spat_w[f,c]
    host-precomputed, scaled by SW=128, clipped to +-240 (TRN e4m3).
  * Moving operand xf8[(k2,c), j, t] = 16*x[c, t+k2+32j] in fp8:
    plane builds via ScalarE/DVE casts + one SBUF shift-DMA.
  * norm_in: x^2 (ScalarE), sum over c via ones-stationary matmul,
    then per-batch chunked layout [16 chunks, 256+64 halo] so the
    sliding-window-64 doubling shifts + reciprocal run on short rows.
  * epilogue per (b, t-tile): broadcast 1/norm row across 128 partitions
    with a K=1 matmul; |conved| via ScalarE Abs; fused multiply+row-sum
    on DVE (accum_out).
  * finish: S[f,b] sums -> one matmul with stationary
    g[f] = 64*weight/(4033*norm_w*SW*SX) contracting over f, + sum(bias).
"""

import contextlib
import ctypes
import sys
import types

import numpy as np

import concourse.bass as bass
import concourse.mybir as mybir
import concourse.tile as tile
from concourse.bass_utils import run_bass_kernel_spmd
from concourse.vector_clock import ScopedClock

F32 = mybir.dt.float32
F32R = mybir.dt.float32r
BF16 = mybir.dt.bfloat16
F8 = mybir.dt.float8e4

B, CIN, T = 64, 64, 4096
F, K = 128, 64
TOUT = T - K + 1          # 4033
NCORES = 8
BLOC = B // NCORES        # 8 batches per core
KQ = 16                   # DoubleRow quad groups: k = 2*kq + k2 + 32*j
TS = 512                  # moving-operand tile (one fp32 PSUM bank)
NTILES = (TOUT + TS - 1) // TS      # 8 (last tile 449)
SCALE = 64.0              # sqrt(CIN*K)
SW = 128.0                # fp8 weight scale
SX = 1.0                  # fp8 x scale (cast-DMA cannot scale)
NCH = 16                  # norm chunks per batch
CW = 256                  # chunk output width
CPAD = 320                # chunk width incl. 64-halo
WAVE = 3                  # conv tiles accumulating concurrently (PSUM banks)

AF = mybir.ActivationFunctionType
ALU = mybir.AluOpType
DR = mybir.MatmulPerfMode.DoubleRow


# ---------------------------------------------------------------------------
# Container fixups: walrus here rejects >1 sem-wait on a Drain; TileContext's
# tail drain carries one wait per logical processor.  Chunk into single-wait
# drains.  Also recreate the (absent) antenv.axon_hooks NTFF profile hook so
# trace=True works when a test harness wants timings.
# ---------------------------------------------------------------------------

def _patched_drain_and_barrier(self, tick_clock, wait_clock):
    nc = self.nc
    drain_inst = nc.sync.drain()
    wait_clock.add_sem_waits(
        drain_inst.ins, ScopedClock({None: tick_clock.global_clock})
    )
    si = drain_inst.ins.sync_info
    waits = list(si.on_wait or []) if si else []
    if len(waits) > 1:
        si.on_wait = waits[:1]
        for w in waits[1:]:
            d2 = nc.sync.drain()
            si2 = d2.ins.sync_info
            if si2 is None:
                d2.ins.sync_info = mybir.SyncInfo(on_wait=[w], on_update=[])
            else:
                si2.on_wait = [w]
    nc.all_engine_barrier()
    assert self.sems is not None
    popped = nc._tile_sem_poison_stack.pop()
    assert popped is self._sem_poison
    nc.clear_and_free_semaphores(list(self.sems.allocated().values()))
    nc.all_engine_barrier()


def _install_ntff_hook():
    if "antenv.axon_hooks" in sys.modules:
        return
    try:
        lib = ctypes.CDLL("/opt/axon/libaxon_pjrt.so")
    except OSError:
        return
    if not hasattr(lib, "axon_start_nrt_profile"):
        return
    lib.axon_start_nrt_profile.argtypes = [
        ctypes.POINTER(ctypes.c_int64),
        ctypes.c_size_t,
    ]
    lib.axon_start_nrt_profile.restype = ctypes.c_int64
    lib.axon_stop_nrt_profile.argtypes = [ctypes.c_char_p]
    lib.axon_stop_nrt_profile.restype = ctypes.c_int64

    @contextlib.contextmanager
    def _hook(output_dir, device_ids):
        import jax

        jax.devices()
        if device_ids:
            ids = (ctypes.c_int64 * len(device_ids))(*device_ids)
            rc = lib.axon_start_nrt_profile(ids, len(device_ids))
        else:
            rc = lib.axon_start_nrt_profile(None, 0)
        if rc != 0:
            raise RuntimeError(f"axon_start_nrt_profile rc={rc}")
        try:
            yield
        finally:
            n = lib.axon_stop_nrt_profile(str(output_dir).encode())
            print(f"profile: {n} ntff file(s) in {output_dir}", file=sys.stderr)

    mod = types.ModuleType("antenv.axon_hooks")
    mod.get_axon_ntff_profile_hook = lambda: _hook
    mod.set_axon_ntff_profile_hook = lambda h: None
    import antenv

    antenv.axon_hooks = mod
    sys.modules["antenv.axon_hooks"] = mod


_ORIG_COMMIT = tile.TileContext._commit_instruction


def _commit_split_waits(self, inst, lazy_reg_writes=True):
    """walrus here allows only one sem-wait per instruction; move extras
    onto same-engine NOPs committed immediately before the instruction."""
    si = getattr(inst, "sync_info", None)
    if (
        si is not None
        and si.on_wait
        and len(si.on_wait) > 1
        and inst.engine != mybir.EngineType.Unassigned
    ):
        waits = list(si.on_wait)
        si.on_wait = waits[:1]
        for i, w in enumerate(waits[1:]):
            nop = mybir.InstNoOp(
                name=f"{inst.name}-wsplit{i}", ins=[], outs=[]
            )
            nop.engine = inst.engine
            nop.sync_info = mybir.SyncInfo(on_wait=[w], on_update=[])
            _ORIG_COMMIT(self, nop, lazy_reg_writes=False)
    return _ORIG_COMMIT(self, inst, lazy_reg_writes)


def install_fixups():
    tile.TileContext._drain_and_barrier = _patched_drain_and_barrier
    tile.TileContext._commit_instruction = _commit_split_waits
    _install_ntff_hook()


# ---------------------------------------------------------------------------
# Device program (identical on all 8 cores; inputs differ per core)
# ---------------------------------------------------------------------------

def build_program() -> bass.Bass:
    install_fixups()
    nc = bass.Bass()

    xs_in = nc.dram_tensor("xs", [BLOC, CIN, T], F32, kind="ExternalInput")
    wdr_in = nc.dram_tensor("wdr", [128, KQ, 2, F], F8, kind="ExternalInput")
    g_in = nc.dram_tensor("g", [F, 1], F32, kind="ExternalInput")
    bsum_in = nc.dram_tensor("bsum", [1, 1], F32, kind="ExternalInput")
    ones64_in = nc.dram_tensor("ones64", [64, 1], BF16, kind="ExternalInput")
    out_d = nc.dram_tensor("out", [1, BLOC], F32, kind="ExternalOutput")
    rdram = nc.dram_tensor("rdram", [BLOC, NCH, CW], BF16, kind="Internal")

    PS = bass.MemorySpace.PSUM

    with tile.TileContext(nc) as tc:
        with (
            tc.tile_pool(name="const", bufs=1) as constp,
            tc.tile_pool(name="xtp", bufs=2) as xtp,
            tc.tile_pool(name="sqp", bufs=2) as sqp,
            tc.tile_pool(name="xf8p", bufs=2) as xf8p,
            tc.tile_pool(name="rowp", bufs=2) as rowp,
            tc.tile_pool(name="slidep", bufs=3) as slidep,
            tc.tile_pool(name="rcp", bufs=2) as rcp,
            tc.tile_pool(name="pbsp", bufs=3) as pbsp,
            tc.tile_pool(name="scrp", bufs=3) as scrp,
            tc.tile_pool(name="accp", bufs=2) as accp,
            tc.tile_pool(name="pconv", bufs=2 * WAVE, space=PS) as pconv,
            tc.tile_pool(name="psq", bufs=1, space=PS) as psq,
        ):
            # constants
            wdr = constp.tile([128, KQ, 2, F], F8)
            nc.sync.dma_start(wdr[:], wdr_in[:])
            gsb = constp.tile([F, 1], F32)
            nc.sync.dma_start(gsb[:], g_in[:])
            bsumsb = constp.tile([1, 1], F32)
            nc.sync.dma_start(bsumsb[:], bsum_in[:])
            ones64 = constp.tile([64, 1], BF16)
            nc.sync.dma_start(ones64[:], ones64_in[:])

            S = constp.tile([F, BLOC], F32)        # per-(f,b) |cos| sums

            for b in range(BLOC):
                # ---- load + squares -------------------------------------
                xt = xtp.tile([64, T], F32, tag="xt")
                nc.sync.dma_start(xt[:], xs_in[b])
                xsq = sqp.tile([64, T], BF16, tag="xsq")
                nc.scalar.activation(xsq[:], xt[:], AF.Square)

                # sum over c via ones-stationary matmuls; write rows into
                # the chunk-major staging row [1, 17, 256] (pad chunk = 1.0)
                srow = rowp.tile([1, NCH + 1, CW], BF16, tag="srow")
                nc.vector.memset(srow[0:1, NCH, :], 1.0)
                for ts in range(T // TS):
                    pq = psq.tile([1, TS], F32)
                    nc.tensor.matmul(
                        pq[:],
                        ones64[:],
                        xsq[:, ts * TS:(ts + 1) * TS],
                    )
                    nc.scalar.copy(srow[0:1, 2 * ts:2 * ts + 2, :], pq[:])

                # chunked [16, 320] = [body 256 | halo 64] via reshape DMAs
                sqc = slidep.tile([NCH, CPAD], BF16, tag="slide")
                nc.sync.dma_start(sqc[:, 0:CW], srow[0:1, 0:NCH, :])
                nc.sync.dma_start(sqc[:, CW:CPAD], srow[0:1, 1:NCH + 1, 0:64])

                # sliding-window-64 sum via doubling shifts, then 1/sqrt
                cur = sqc
                width = CPAD
                for sh in (1, 2, 4, 8, 16, 32):
                    width -= sh
                    nxt = slidep.tile([NCH, CPAD], BF16, tag="slide")
                    nc.vector.tensor_tensor(
                        nxt[:, 0:width], cur[:, 0:width], cur[:, sh:sh + width],
                        op=ALU.add,
                    )
                    cur = nxt
                assert width == CW + 1
                rec = rowp.tile([NCH, CPAD], F32, tag="rec")
                nc.vector.reciprocal(rec[:, 0:CW], cur[:, 0:CW])
                rc = rcp.tile([NCH, CW], BF16, tag="rc")
                nc.scalar.activation(rc[:], rec[:, 0:CW], AF.Sqrt)
                nc.sync.dma_start(rdram[b], rc[:])

                # ---- fp8 moving operand (SWDGE cast-DMAs) ---------------
                # xf8[(k2,c), j, t] = x[c, t + k2 + 32j]
                xf8 = xf8p.tile([128, 2, T], F8, tag="xf8")
                nc.vector.memset(xf8[:, 0:2, T - 64:T], 0.0)
                nc.gpsimd.dma_start(xf8[0:64, 0, 0:T], xt[:])
                nc.gpsimd.dma_start(xf8[0:64, 1, 0:T - 32], xt[0:64, 32:T])
                nc.sync.dma_start(
                    xf8[64:128, 0:2, 0:T - 1], xf8[0:64, 0:2, 1:T]
                )

                # ---- conv + cosine epilogue (waves of WAVE tiles) -------
                acc = accp.tile([F, NTILES], F32)
                for w0 in range(0, NTILES, WAVE):
                    wtiles = list(range(w0, min(w0 + WAVE, NTILES)))
                    pcs = {}
                    for ts in wtiles:
                        pcs[ts] = pconv.tile(
                            [F, TS], F32, name=f"pc_{b}_{ts}", tag="pc"
                        )
                    # kq-outer: one stationary serves len(wtiles) matmuls
                    for kq in range(KQ):
                        for ts in wtiles:
                            t0 = ts * TS
                            nt = min(TS, TOUT - t0)
                            nt_mm = nt + (nt & 1)
                            nc.tensor.matmul(
                                pcs[ts][:, 0:nt_mm],
                                wdr[:, kq, 0:2, :],
                                xf8[:, 0:2, t0 + 2 * kq: t0 + 2 * kq + nt_mm],
                                start=(kq == 0),
                                stop=(kq == KQ - 1),
                                perf_mode=DR,
                            )
                    for ts in wtiles:
                        t0 = ts * TS
                        nt = min(TS, TOUT - t0)
                        # 1/norm row for this tile, broadcast across
                        # partitions by DMA from the DRAM copy (not PE)
                        pbs = pbsp.tile([128, TS], BF16, tag="pbs")
                        nc.sync.dma_start(
                            pbs[:],
                            rdram[b, 2 * ts:2 * ts + 2, 0:CW]
                            .partition_broadcast(128),
                        )
                        # DVE may read only one PSUM operand: take |conv| on
                        # ScalarE (PSUM->SBUF), then multiply+row-sum on DVE.
                        scr = scrp.tile([F, TS], BF16, tag="scr")
                        nc.scalar.activation(
                            scr[:, 0:nt], pcs[ts][:, 0:nt], AF.Abs
                        )
                        nc.vector.scalar_tensor_tensor(
                            scr[:, 0:nt],
                            scr[:, 0:nt],
                            1.0,
                            pbs[:, 0:nt],
                            op0=ALU.mult,
                            op1=ALU.mult,
                            accum_out=acc[:, ts:ts + 1],
                        )
                nc.vector.reduce_sum(
                    S[:, b:b + 1], acc[:], axis=mybir.AxisListType.X
                )

            # ---- finish: out[b] = sum_f g[f]*S[f,b] + sum_f bias[f] ------
            pf = pconv.tile([1, BLOC], F32, name="pf", tag="pc")
            nc.tensor.matmul(pf[:], gsb[:], S[:])
            out_sb = constp.tile([1, BLOC], F32)
            nc.scalar.add(out_sb[:], pf[:], bsumsb[0:1, 0:1])
            nc.sync.dma_start(out_d[:], out_sb[:])

    return nc


_PROGRAM: bass.Bass | None = None


def _get_program() -> bass.Bass:
    global _PROGRAM
    if _PROGRAM is None:
        _PROGRAM = build_program()
    return _PROGRAM


# ---------------------------------------------------------------------------
# Host entry point
# ---------------------------------------------------------------------------

def host_params(conv_weights, spat_weights, weight, bias):
    """Tiny host-side precomputation of stationaries and scalars."""
    conv = np.asarray(conv_weights, dtype=np.float64)
    spat = np.asarray(spat_weights, dtype=np.float64)
    w = np.asarray(weight, dtype=np.float64)
    bb = np.asarray(bias, dtype=np.float64)

    # prod[k, c, f] = conv[f, k] * spat[f, c]; DoubleRow packing
    # k = 32*j + 2*kq + k2  ->  wdr[(k2,c), kq, j, f]
    prod = np.einsum("fk,fc->kcf", conv, spat) * SW    # [K, C, F]
    P5 = prod.reshape(2, KQ, 2, CIN, F)                # [j, kq, k2, c, f]
    wdr = np.ascontiguousarray(P5.transpose(2, 3, 1, 0, 4)).reshape(
        128, KQ, 2, F
    )
    wdr = np.clip(wdr, -240.0, 240.0)
    f8np = mybir.dt.np(F8)
    wdr = wdr.astype(np.float32).astype(f8np)

    norm_w = np.sqrt((spat * spat).sum(1) * (conv * conv).sum(1))  # [F]
    g = (SCALE / (TOUT * norm_w * SW * SX) * w).astype(np.float32).reshape(F, 1)
    bsum = np.array([[bb.sum()]], dtype=np.float32)
    return wdr, g, bsum


def make_in_maps(x, conv_weights, spat_weights, weight, bias):
    x = np.ascontiguousarray(np.asarray(x, dtype=np.float32))
    wdr, g, bsum = host_params(conv_weights, spat_weights, weight, bias)
    bf16np = mybir.dt.np(BF16)
    in_maps = []
    for c in range(NCORES):
        in_maps.append(
            {
                "xs": np.ascontiguousarray(x[c * BLOC:(c + 1) * BLOC]),
                "wdr": wdr,
                "g": g,
                "bsum": bsum,
                "ones64": np.ones((64, 1), bf16np),
            }
        )
    return in_maps


def kernel(x, conv_weights, spat_weights, weight, bias):
    in_maps = make_in_maps(x, conv_weights, spat_weights, weight, bias)
    nc = _get_program()
    res = run_bass_kernel_spmd(nc, in_maps, core_ids=list(range(NCORES)))
    out = np.concatenate(
        [res.results[c]["out"].reshape(BLOC) for c in range(NCORES)]
    )
    return out.astype(np.float32)


# revision 15
# speedup vs baseline: 2.1139x; 1.0603x over previous
"""Trainium2 Bass kernel for nn_CosSimSpatTempConvNet.

Math (reference):
  merged[f,c,k] = conv_w[f,k] * spat_w[f,c]                  (rank-1 kernel)
  conved[b,f,t] = sum_{c,k} merged[f,c,k] * x[b,c,t+k]       (valid conv, Tout=T-K+1)
  norm_w[f]    = ||conv_w[f]|| * ||spat_w[f]||
  norm_in[b,t] = sqrt(sum_{c,k} x[b,c,t+k]^2)
  cos[b,f,t]   = conved * 64 / (norm_w[f] * norm_in[b,t])
  out[b]       = sum_f (mean_t |cos[b,f,t]| * weight[f] + bias[f])

Device strategy (8 cores, data-parallel over batch, 8 b per core):
  * Conv as fp8e4 DoubleRow TensorE matmuls: contraction 256 per
    instruction = 128 partitions (k2 in {0,1} x c) x 2 k-planes
    (j in {0,1}; k = 2*kq + k2 + 32*j).  16 PSUM-accumulated matmuls per
    512-wide time tile (vs 32 for fp32r), 0.5 cycles/output-row.
    Stationaries wdr[(k2,c), kq, j, f] = conv_w[f,2kq+k2+32j]*spat_w[f,c]
    host-precomputed, scaled by SW=128, clipped to +-240 (TRN e4m3).
  * Moving operand xf8[(k2,c), j, t] = 16*x[c, t+k2+32j] in fp8:
    plane builds via ScalarE/DVE casts + one SBUF shift-DMA.
  * norm_in: x^2 (ScalarE), sum over c via ones-stationary matmul,
    then per-batch chunked layout [16 chunks, 256+64 halo] so the
    sliding-window-64 doubling shifts + reciprocal run on short rows.
  * epilogue per (b, t-tile): broadcast 1/norm row across 128 partitions
    with a K=1 matmul; |conved| via ScalarE Abs; fused multiply+row-sum
    on DVE (accum_out).
  * finish: S[f,b] sums -> one matmul with stationary
    g[f] = 64*weight/(4033*norm_w*SW*SX) contracting over f, + sum(bias).
"""

import contextlib
import ctypes
import sys
import types

import numpy as np

import concourse.bass as bass
import concourse.mybir as mybir
import concourse.tile as tile
from concourse.bass_utils import run_bass_kernel_spmd
from concourse.vector_clock import ScopedClock

F32 = mybir.dt.float32
F32R = mybir.dt.float32r
BF16 = mybir.dt.bfloat16
F8 = mybir.dt.float8e4

B, CIN, T = 64, 64, 4096
F, K = 128, 64
TOUT = T - K + 1          # 4033
NCORES = 8
BLOC = B // NCORES        # 8 batches per core
KQ = 16                   # DoubleRow quad groups: k = 2*kq + k2 + 32*j
TS = 512                  # moving-operand tile (one fp32 PSUM bank)
NTILES = (TOUT + TS - 1) // TS      # 8 (last tile 449)
SCALE = 64.0              # sqrt(CIN*K)
SW = 128.0                # fp8 weight scale
SX = 1.0                  # fp8 x scale (cast-DMA cannot scale)
NCH = 16                  # norm chunks per batch
CW = 256                  # chunk output width
CPAD = 320                # chunk width incl. 64-halo
WAVE = 3                  # conv tiles accumulating concurrently (PSUM banks)

AF = mybir.ActivationFunctionType
ALU = mybir.AluOpType
DR = mybir.MatmulPerfMode.DoubleRow


# ---------------------------------------------------------------------------
# Container fixups: walrus here rejects >1 sem-wait on a Drain; TileContext's
# tail drain carries one wait per logical processor.  Chunk into single-wait
# drains.  Also recreate the (absent) antenv.axon_hooks NTFF profile hook so
# trace=True works when a test harness wants timings.
# ---------------------------------------------------------------------------

def _patched_drain_and_barrier(self, tick_clock, wait_clock):
    nc = self.nc
    drain_inst = nc.sync.drain()
    wait_clock.add_sem_waits(
        drain_inst.ins, ScopedClock({None: tick_clock.global_clock})
    )
    si = drain_inst.ins.sync_info
    waits = list(si.on_wait or []) if si else []
    if len(waits) > 1:
        si.on_wait = waits[:1]
        for w in waits[1:]:
            d2 = nc.sync.drain()
            si2 = d2.ins.sync_info
            if si2 is None:
                d2.ins.sync_info = mybir.SyncInfo(on_wait=[w], on_update=[])
            else:
                si2.on_wait = [w]
    nc.all_engine_barrier()
    assert self.sems is not None
    popped = nc._tile_sem_poison_stack.pop()
    assert popped is self._sem_poison
    nc.clear_and_free_semaphores(list(self.sems.allocated().values()))
    nc.all_engine_barrier()


def _install_ntff_hook():
    if "antenv.axon_hooks" in sys.modules:
        return
    try:
        lib = ctypes.CDLL("/opt/axon/libaxon_pjrt.so")
    except OSError:
        return
    if not hasattr(lib, "axon_start_nrt_profile"):
        return
    lib.axon_start_nrt_profile.argtypes = [
        ctypes.POINTER(ctypes.c_int64),
        ctypes.c_size_t,
    ]
    lib.axon_start_nrt_profile.restype = ctypes.c_int64
    lib.axon_stop_nrt_profile.argtypes = [ctypes.c_char_p]
    lib.axon_stop_nrt_profile.restype = ctypes.c_int64

    @contextlib.contextmanager
    def _hook(output_dir, device_ids):
        import jax

        jax.devices()
        if device_ids:
            ids = (ctypes.c_int64 * len(device_ids))(*device_ids)
            rc = lib.axon_start_nrt_profile(ids, len(device_ids))
        else:
            rc = lib.axon_start_nrt_profile(None, 0)
        if rc != 0:
            raise RuntimeError(f"axon_start_nrt_profile rc={rc}")
        try:
            yield
        finally:
            n = lib.axon_stop_nrt_profile(str(output_dir).encode())
            print(f"profile: {n} ntff file(s) in {output_dir}", file=sys.stderr)

    mod = types.ModuleType("antenv.axon_hooks")
    mod.get_axon_ntff_profile_hook = lambda: _hook
    mod.set_axon_ntff_profile_hook = lambda h: None
    import antenv

    antenv.axon_hooks = mod
    sys.modules["antenv.axon_hooks"] = mod


_ORIG_COMMIT = tile.TileContext._commit_instruction


def _commit_split_waits(self, inst, lazy_reg_writes=True):
    """walrus here allows only one sem-wait per instruction; move extras
    onto same-engine NOPs committed immediately before the instruction."""
    si = getattr(inst, "sync_info", None)
    if (
        si is not None
        and si.on_wait
        and len(si.on_wait) > 1
        and inst.engine != mybir.EngineType.Unassigned
    ):
        waits = list(si.on_wait)
        si.on_wait = waits[:1]
        for i, w in enumerate(waits[1:]):
            nop = mybir.InstNoOp(
                name=f"{inst.name}-wsplit{i}", ins=[], outs=[]
            )
            nop.engine = inst.engine
            nop.sync_info = mybir.SyncInfo(on_wait=[w], on_update=[])
            _ORIG_COMMIT(self, nop, lazy_reg_writes=False)
    return _ORIG_COMMIT(self, inst, lazy_reg_writes)


def install_fixups():
    tile.TileContext._drain_and_barrier = _patched_drain_and_barrier
    tile.TileContext._commit_instruction = _commit_split_waits
    _install_ntff_hook()


# ---------------------------------------------------------------------------
# Device program (identical on all 8 cores; inputs differ per core)
# ---------------------------------------------------------------------------

def build_program() -> bass.Bass:
    install_fixups()
    nc = bass.Bass()

    xs_in = nc.dram_tensor("xs", [BLOC, CIN, T], F32, kind="ExternalInput")
    wdr_in = nc.dram_tensor("wdr", [128, KQ, 2, F], F8, kind="ExternalInput")
    g_in = nc.dram_tensor("g", [F, 1], F32, kind="ExternalInput")
    bsum_in = nc.dram_tensor("bsum", [1, 1], F32, kind="ExternalInput")
    ones64_in = nc.dram_tensor("ones64", [64, 1], BF16, kind="ExternalInput")
    out_d = nc.dram_tensor("out", [1, BLOC], F32, kind="ExternalOutput")
    rdram = nc.dram_tensor("rdram", [BLOC, NCH, CW], BF16, kind="Internal")

    PS = bass.MemorySpace.PSUM

    with tile.TileContext(nc) as tc:
        with (
            tc.tile_pool(name="const", bufs=1) as constp,
            tc.tile_pool(name="xtp", bufs=2) as xtp,
            tc.tile_pool(name="sqp", bufs=2) as sqp,
            tc.tile_pool(name="xf8p", bufs=2) as xf8p,
            tc.tile_pool(name="rowp", bufs=2) as rowp,
            tc.tile_pool(name="slidep", bufs=3) as slidep,
            tc.tile_pool(name="rcp", bufs=2) as rcp,
            tc.tile_pool(name="pbsp", bufs=3) as pbsp,
            tc.tile_pool(name="scrp", bufs=3) as scrp,
            tc.tile_pool(name="accp", bufs=2) as accp,
            tc.tile_pool(name="pconv", bufs=2 * WAVE, space=PS) as pconv,
            tc.tile_pool(name="psq", bufs=2, space=PS) as psq,
        ):
            # constants
            wdr = constp.tile([128, KQ, 2, F], F8)
            nc.sync.dma_start(wdr[:], wdr_in[:])
            gsb = constp.tile([F, 1], F32)
            nc.sync.dma_start(gsb[:], g_in[:])
            bsumsb = constp.tile([1, 1], F32)
            nc.sync.dma_start(bsumsb[:], bsum_in[:])
            ones64 = constp.tile([64, 1], BF16)
            nc.sync.dma_start(ones64[:], ones64_in[:])

            S = constp.tile([F, BLOC], F32)        # per-(f,b) |cos| sums

            def load_batch(b):
                """DMA x[b], build fp8 moving operand, square for norms."""
                xt = xtp.tile([64, T], F32, tag="xt", name=f"xt{b}")
                nc.sync.dma_start(xt[:], xs_in[b])
                # xf8[(k2,c), j, t] = x[c, t + k2 + 32j] via SWDGE cast-DMAs
                xf8 = xf8p.tile([128, 2, T], F8, tag="xf8", name=f"xf8_{b}")
                nc.vector.memset(xf8[:, 0:2, T - 64:T], 0.0)
                nc.gpsimd.dma_start(xf8[0:64, 0, 0:T], xt[:])
                nc.gpsimd.dma_start(xf8[0:64, 1, 0:T - 32], xt[0:64, 32:T])
                nc.sync.dma_start(
                    xf8[64:128, 0:2, 0:T - 1], xf8[0:64, 0:2, 1:T]
                )
                xsq = sqp.tile([64, T], BF16, tag="xsq", name=f"xsq{b}")
                nc.scalar.activation(xsq[:], xt[:], AF.Square)
                srow = rowp.tile(
                    [1, NCH + 1, CW], BF16, tag="srow", name=f"srow{b}"
                )
                nc.vector.memset(srow[0:1, NCH, :], 1.0)
                return {"xf8": xf8, "xsq": xsq, "srow": srow}

            def sq_mms(st, tslist):
                """Per-c-sum matmuls (interleavable between conv waves)."""
                for ts in tslist:
                    pq = psq.tile([1, TS], F32, name="pq", tag="pq")
                    nc.tensor.matmul(
                        pq[:],
                        ones64[:],
                        st["xsq"][:, ts * TS:(ts + 1) * TS],
                    )
                    nc.scalar.copy(
                        st["srow"][0:1, 2 * ts:2 * ts + 2, :], pq[:]
                    )

            def norm_chain(b, st):
                """Sliding-window-64 sums + 1/sqrt; result to rdram[b]."""
                srow = st["srow"]
                sqc = slidep.tile([NCH, CPAD], BF16, tag="slide",
                                  name=f"sqc{b}")
                nc.sync.dma_start(sqc[:, 0:CW], srow[0:1, 0:NCH, :])
                nc.sync.dma_start(
                    sqc[:, CW:CPAD], srow[0:1, 1:NCH + 1, 0:64]
                )
                cur = sqc
                width = CPAD
                for sh in (1, 2, 4, 8, 16, 32):
                    width -= sh
                    nxt = slidep.tile([NCH, CPAD], BF16, tag="slide",
                                      name=f"sl{b}_{sh}")
                    nc.vector.tensor_tensor(
                        nxt[:, 0:width], cur[:, 0:width],
                        cur[:, sh:sh + width], op=ALU.add,
                    )
                    cur = nxt
                assert width == CW + 1
                rec = rowp.tile([NCH, CPAD], F32, tag="rec", name=f"rec{b}")
                nc.vector.reciprocal(rec[:, 0:CW], cur[:, 0:CW])
                rc = rcp.tile([NCH, CW], BF16, tag="rc", name=f"rc{b}")
                nc.scalar.activation(rc[:], rec[:, 0:CW], AF.Sqrt)
                nc.sync.dma_start(rdram[b], rc[:])

            def conv_batch(b, st, inter_st):
                """Conv waves + cosine epilogue; sq matmuls of the *next*
                batch (inter_st) are interleaved between waves to keep the
                PE stream dense."""
                xf8 = st["xf8"]
                acc = accp.tile([F, NTILES], F32, name=f"acc{b}")
                sq_chunks = [(0, 3), (3, 6), (6, 8)]
                for wi, w0 in enumerate(range(0, NTILES, WAVE)):
                    wtiles = list(range(w0, min(w0 + WAVE, NTILES)))
                    pcs = {}
                    for ts in wtiles:
                        pcs[ts] = pconv.tile(
                            [F, TS], F32, name=f"pc_{b}_{ts}", tag="pc"
                        )
                    # kq-outer: one stationary serves len(wtiles) matmuls;
                    # followers skip the redundant LDWEIGHTS
                    for kq in range(KQ):
                        for i, ts in enumerate(wtiles):
                            t0 = ts * TS
                            nt = min(TS, TOUT - t0)
                            nt_mm = nt + (nt & 1)
                            bi = nc.tensor.matmul(
                                pcs[ts][:, 0:nt_mm],
                                wdr[:, kq, 0:2, :],
                                xf8[:, 0:2, t0 + 2 * kq: t0 + 2 * kq + nt_mm],
                                start=(kq == 0),
                                stop=(kq == KQ - 1),
                                perf_mode=DR,
                            )
                            if i > 0:
                                bi.ins.ldweights = False
                    if inter_st is not None:
                        lo, hi = sq_chunks[wi]
                        sq_mms(inter_st, range(lo, hi))
                    for ts in wtiles:
                        t0 = ts * TS
                        nt = min(TS, TOUT - t0)
                        # 1/norm row, broadcast across partitions by DMA
                        # from the DRAM copy (not PE)
                        pbs = pbsp.tile(
                            [128, TS], BF16, tag="pbs", name=f"pbs{b}_{ts}"
                        )
                        nc.sync.dma_start(
                            pbs[:],
                            rdram[b, 2 * ts:2 * ts + 2, 0:CW]
                            .partition_broadcast(128),
                        )
                        # DVE may read only one PSUM operand: take |conv| on
                        # ScalarE (PSUM->SBUF), then multiply+row-sum on DVE.
                        scr = scrp.tile(
                            [F, TS], BF16, tag="scr", name=f"scr{b}_{ts}"
                        )
                        nc.scalar.activation(
                            scr[:, 0:nt], pcs[ts][:, 0:nt], AF.Abs
                        )
                        nc.vector.scalar_tensor_tensor(
                            scr[:, 0:nt],
                            scr[:, 0:nt],
                            1.0,
                            pbs[:, 0:nt],
                            op0=ALU.mult,
                            op1=ALU.mult,
                            accum_out=acc[:, ts:ts + 1],
                        )
                nc.vector.reduce_sum(
                    S[:, b:b + 1], acc[:], axis=mybir.AxisListType.X
                )

            # software pipeline: batch b+1's load/squares/norms overlap
            # batch b's conv
            st = load_batch(0)
            sq_mms(st, range(NTILES))
            norm_chain(0, st)
            for b in range(BLOC):
                nxt_st = load_batch(b + 1) if b + 1 < BLOC else None
                conv_batch(b, st, nxt_st)
                if nxt_st is not None:
                    norm_chain(b + 1, nxt_st)
                st = nxt_st

            # ---- finish: out[b] = sum_f g[f]*S[f,b] + sum_f bias[f] ------
            pf = pconv.tile([1, BLOC], F32, name="pf", tag="pc")
            nc.tensor.matmul(pf[:], gsb[:], S[:])
            out_sb = constp.tile([1, BLOC], F32)
            nc.scalar.add(out_sb[:], pf[:], bsumsb[0:1, 0:1])
            nc.sync.dma_start(out_d[:], out_sb[:])

    return nc


_PROGRAM: bass.Bass | None = None


def _get_program() -> bass.Bass:
    global _PROGRAM
    if _PROGRAM is None:
        _PROGRAM = build_program()
    return _PROGRAM


# ---------------------------------------------------------------------------
# Host entry point
# ---------------------------------------------------------------------------

def host_params(conv_weights, spat_weights, weight, bias):
    """Tiny host-side precomputation of stationaries and scalars."""
    conv = np.asarray(conv_weights, dtype=np.float64)
    spat = np.asarray(spat_weights, dtype=np.float64)
    w = np.asarray(weight, dtype=np.float64)
    bb = np.asarray(bias, dtype=np.float64)

    # prod[k, c, f] = conv[f, k] * spat[f, c]; DoubleRow packing
    # k = 32*j + 2*kq + k2  ->  wdr[(k2,c), kq, j, f]
    prod = np.einsum("fk,fc->kcf", conv, spat) * SW    # [K, C, F]
    P5 = prod.reshape(2, KQ, 2, CIN, F)                # [j, kq, k2, c, f]
    wdr = np.ascontiguousarray(P5.transpose(2, 3, 1, 0, 4)).reshape(
        128, KQ, 2, F
    )
    wdr = np.clip(wdr, -240.0, 240.0)
    f8np = mybir.dt.np(F8)
    wdr = wdr.astype(np.float32).astype(f8np)

    norm_w = np.sqrt((spat * spat).sum(1) * (conv * conv).sum(1))  # [F]
    g = (SCALE / (TOUT * norm_w * SW * SX) * w).astype(np.float32).reshape(F, 1)
    bsum = np.array([[bb.sum()]], dtype=np.float32)
    return wdr, g, bsum


def make_in_maps(x, conv_weights, spat_weights, weight, bias):
    x = np.ascontiguousarray(np.asarray(x, dtype=np.float32))
    wdr, g, bsum = host_params(conv_weights, spat_weights, weight, bias)
    bf16np = mybir.dt.np(BF16)
    in_maps = []
    for c in range(NCORES):
        in_maps.append(
            {
                "xs": np.ascontiguousarray(x[c * BLOC:(c + 1) * BLOC]),
                "wdr": wdr,
                "g": g,
                "bsum": bsum,
                "ones64": np.ones((64, 1), bf16np),
            }
        )
    return in_maps


def kernel(x, conv_weights, spat_weights, weight, bias):
    in_maps = make_in_maps(x, conv_weights, spat_weights, weight, bias)
    nc = _get_program()
    res = run_bass_kernel_spmd(nc, in_maps, core_ids=list(range(NCORES)))
    out = np.concatenate(
        [res.results[c]["out"].reshape(BLOC) for c in range(NCORES)]
    )
    return out.astype(np.float32)


# revision 16
# speedup vs baseline: 2.3937x; 1.1324x over previous
"""Trainium2 Bass kernel for nn_CosSimSpatTempConvNet.

Math (reference):
  merged[f,c,k] = conv_w[f,k] * spat_w[f,c]                  (rank-1 kernel)
  conved[b,f,t] = sum_{c,k} merged[f,c,k] * x[b,c,t+k]       (valid conv, Tout=T-K+1)
  norm_w[f]    = ||conv_w[f]|| * ||spat_w[f]||
  norm_in[b,t] = sqrt(sum_{c,k} x[b,c,t+k]^2)
  cos[b,f,t]   = conved * 64 / (norm_w[f] * norm_in[b,t])
  out[b]       = sum_f (mean_t |cos[b,f,t]| * weight[f] + bias[f])

Device strategy (8 cores, data-parallel over batch, 8 b per core):
  * Conv as fp8e4 DoubleRow TensorE matmuls: contraction 256 per
    instruction = 128 partitions (k2 in {0,1} x c) x 2 k-planes
    (j in {0,1}; k = 2*kq + k2 + 32*j).  16 PSUM-accumulated matmuls per
    512-wide time tile (vs 32 for fp32r), 0.5 cycles/output-row.
    Stationaries wdr[(k2,c), kq, j, f] = conv_w[f,2kq+k2+32j]*spat_w[f,c]
    host-precomputed, scaled by SW=128, clipped to +-240 (TRN e4m3).
  * Moving operand xf8[(k2,c), j, t] = 16*x[c, t+k2+32j] in fp8:
    plane builds via ScalarE/DVE casts + one SBUF shift-DMA.
  * norm_in: x^2 (ScalarE), sum over c via ones-stationary matmul,
    then per-batch chunked layout [16 chunks, 256+64 halo] so the
    sliding-window-64 doubling shifts + reciprocal run on short rows.
  * epilogue per (b, t-tile): broadcast 1/norm row across 128 partitions
    with a K=1 matmul; |conved| via ScalarE Abs; fused multiply+row-sum
    on DVE (accum_out).
  * finish: S[f,b] sums -> one matmul with stationary
    g[f] = 64*weight/(4033*norm_w*SW*SX) contracting over f, + sum(bias).
"""

import contextlib
import ctypes
import sys
import types

import numpy as np

import concourse.bass as bass
import concourse.mybir as mybir
import concourse.tile as tile
from concourse.bass_utils import run_bass_kernel_spmd
from concourse.vector_clock import ScopedClock

F32 = mybir.dt.float32
F32R = mybir.dt.float32r
BF16 = mybir.dt.bfloat16
F8 = mybir.dt.float8e4

B, CIN, T = 64, 64, 4096
F, K = 128, 64
TOUT = T - K + 1          # 4033
NCORES = 8
BLOC = B // NCORES        # 8 batches per core
KQ = 16                   # DoubleRow quad groups: k = 2*kq + k2 + 32*j
TS = 512                  # moving-operand tile (one fp32 PSUM bank)
NTILES = (TOUT + TS - 1) // TS      # 8 (last tile 449)
SCALE = 64.0              # sqrt(CIN*K)
SW = 128.0                # fp8 weight scale
SX = 16.0                 # fp8 x scale
NCH = 16                  # norm chunks per batch
CW = 256                  # chunk output width
CPAD = 320                # chunk width incl. 64-halo
WAVE = 3                  # conv tiles accumulating concurrently (PSUM banks)

AF = mybir.ActivationFunctionType
ALU = mybir.AluOpType
DR = mybir.MatmulPerfMode.DoubleRow


# ---------------------------------------------------------------------------
# Container fixups: walrus here rejects >1 sem-wait on a Drain; TileContext's
# tail drain carries one wait per logical processor.  Chunk into single-wait
# drains.  Also recreate the (absent) antenv.axon_hooks NTFF profile hook so
# trace=True works when a test harness wants timings.
# ---------------------------------------------------------------------------

def _patched_drain_and_barrier(self, tick_clock, wait_clock):
    nc = self.nc
    drain_inst = nc.sync.drain()
    wait_clock.add_sem_waits(
        drain_inst.ins, ScopedClock({None: tick_clock.global_clock})
    )
    si = drain_inst.ins.sync_info
    waits = list(si.on_wait or []) if si else []
    if len(waits) > 1:
        si.on_wait = waits[:1]
        for w in waits[1:]:
            d2 = nc.sync.drain()
            si2 = d2.ins.sync_info
            if si2 is None:
                d2.ins.sync_info = mybir.SyncInfo(on_wait=[w], on_update=[])
            else:
                si2.on_wait = [w]
    nc.all_engine_barrier()
    assert self.sems is not None
    popped = nc._tile_sem_poison_stack.pop()
    assert popped is self._sem_poison
    nc.clear_and_free_semaphores(list(self.sems.allocated().values()))
    nc.all_engine_barrier()


def _install_ntff_hook():
    if "antenv.axon_hooks" in sys.modules:
        return
    try:
        lib = ctypes.CDLL("/opt/axon/libaxon_pjrt.so")
    except OSError:
        return
    if not hasattr(lib, "axon_start_nrt_profile"):
        return
    lib.axon_start_nrt_profile.argtypes = [
        ctypes.POINTER(ctypes.c_int64),
        ctypes.c_size_t,
    ]
    lib.axon_start_nrt_profile.restype = ctypes.c_int64
    lib.axon_stop_nrt_profile.argtypes = [ctypes.c_char_p]
    lib.axon_stop_nrt_profile.restype = ctypes.c_int64

    @contextlib.contextmanager
    def _hook(output_dir, device_ids):
        import jax

        jax.devices()
        if device_ids:
            ids = (ctypes.c_int64 * len(device_ids))(*device_ids)
            rc = lib.axon_start_nrt_profile(ids, len(device_ids))
        else:
            rc = lib.axon_start_nrt_profile(None, 0)
        if rc != 0:
            raise RuntimeError(f"axon_start_nrt_profile rc={rc}")
        try:
            yield
        finally:
            n = lib.axon_stop_nrt_profile(str(output_dir).encode())
            print(f"profile: {n} ntff file(s) in {output_dir}", file=sys.stderr)

    mod = types.ModuleType("antenv.axon_hooks")
    mod.get_axon_ntff_profile_hook = lambda: _hook
    mod.set_axon_ntff_profile_hook = lambda h: None
    import antenv

    antenv.axon_hooks = mod
    sys.modules["antenv.axon_hooks"] = mod


_ORIG_COMMIT = tile.TileContext._commit_instruction


def _commit_split_waits(self, inst, lazy_reg_writes=True):
    """walrus here allows only one sem-wait per instruction; move extras
    onto same-engine NOPs committed immediately before the instruction."""
    si = getattr(inst, "sync_info", None)
    if (
        si is not None
        and si.on_wait
        and len(si.on_wait) > 1
        and inst.engine != mybir.EngineType.Unassigned
    ):
        waits = list(si.on_wait)
        si.on_wait = waits[:1]
        for i, w in enumerate(waits[1:]):
            nop = mybir.InstNoOp(
                name=f"{inst.name}-wsplit{i}", ins=[], outs=[]
            )
            nop.engine = inst.engine
            nop.sync_info = mybir.SyncInfo(on_wait=[w], on_update=[])
            _ORIG_COMMIT(self, nop, lazy_reg_writes=False)
    return _ORIG_COMMIT(self, inst, lazy_reg_writes)


def install_fixups():
    tile.TileContext._drain_and_barrier = _patched_drain_and_barrier
    tile.TileContext._commit_instruction = _commit_split_waits
    _install_ntff_hook()


# ---------------------------------------------------------------------------
# Device program (identical on all 8 cores; inputs differ per core)
# ---------------------------------------------------------------------------

def build_program() -> bass.Bass:
    install_fixups()
    nc = bass.Bass()

    xs_in = nc.dram_tensor("xs", [BLOC, CIN, T], F32, kind="ExternalInput")
    wdr_in = nc.dram_tensor("wdr", [128, KQ, 2, F], F8, kind="ExternalInput")
    g_in = nc.dram_tensor("g", [F, 1], F32, kind="ExternalInput")
    bsum_in = nc.dram_tensor("bsum", [1, 1], F32, kind="ExternalInput")
    ones64_in = nc.dram_tensor("ones64", [64, 1], BF16, kind="ExternalInput")
    out_d = nc.dram_tensor("out", [1, BLOC], F32, kind="ExternalOutput")
    rdram = nc.dram_tensor("rdram", [BLOC, NCH, CW], BF16, kind="Internal")

    PS = bass.MemorySpace.PSUM

    with tile.TileContext(nc) as tc:
        with (
            tc.tile_pool(name="const", bufs=1) as constp,
            tc.tile_pool(name="xtp", bufs=3) as xtp,
            tc.tile_pool(name="sqp", bufs=3) as sqp,
            tc.tile_pool(name="xf8p", bufs=3) as xf8p,
            tc.tile_pool(name="rowp", bufs=3) as rowp,
            tc.tile_pool(name="slidep", bufs=3) as slidep,
            tc.tile_pool(name="rcp", bufs=2) as rcp,
            tc.tile_pool(name="pbsp", bufs=3) as pbsp,
            tc.tile_pool(name="scrp", bufs=3) as scrp,
            tc.tile_pool(name="accp", bufs=2) as accp,
            tc.tile_pool(name="pconv", bufs=2 * WAVE, space=PS) as pconv,
            tc.tile_pool(name="psq", bufs=2, space=PS) as psq,
        ):
            # constants
            wdr = constp.tile([128, KQ, 2, F], F8)
            nc.sync.dma_start(wdr[:], wdr_in[:])
            gsb = constp.tile([F, 1], F32)
            nc.sync.dma_start(gsb[:], g_in[:])
            bsumsb = constp.tile([1, 1], F32)
            nc.sync.dma_start(bsumsb[:], bsum_in[:])
            ones64 = constp.tile([64, 1], BF16)
            nc.sync.dma_start(ones64[:], ones64_in[:])

            S = constp.tile([F, BLOC], F32)        # per-(f,b) |cos| sums

            def load_batch(b):
                """DMA x[b], build fp8 moving operand, square for norms."""
                xt = xtp.tile([64, T], F32, tag="xt", name=f"xt{b}")
                nc.sync.dma_start(xt[:], xs_in[b])
                # xf8[(k2,c), j, t] = x[c, t + k2 + 32j] via SWDGE cast-DMAs
                xf8 = xf8p.tile([128, 2, T], F8, tag="xf8", name=f"xf8_{b}")
                nc.vector.memset(xf8[:, 0:2, T - 64:T], 0.0)
                nc.scalar.activation(
                    xf8[0:64, 0, 0:T], xt[:], AF.Copy, scale=SX
                )
                nc.vector.tensor_scalar_mul(
                    xf8[0:64, 1, 0:T - 32], xt[0:64, 32:T], SX
                )
                nc.sync.dma_start(
                    xf8[64:128, 0:2, 0:T - 1], xf8[0:64, 0:2, 1:T]
                )
                xsq = sqp.tile([64, T], BF16, tag="xsq", name=f"xsq{b}")
                nc.scalar.activation(xsq[:], xt[:], AF.Square)
                srow = rowp.tile(
                    [1, NCH + 1, CW], BF16, tag="srow", name=f"srow{b}"
                )
                nc.vector.memset(srow[0:1, NCH, :], 1.0)
                return {"xf8": xf8, "xsq": xsq, "srow": srow}

            def sq_mms(st, tslist):
                """Per-c-sum matmuls (interleavable between conv waves)."""
                for ts in tslist:
                    pq = psq.tile([1, TS], F32, name="pq", tag="pq")
                    nc.tensor.matmul(
                        pq[:],
                        ones64[:],
                        st["xsq"][:, ts * TS:(ts + 1) * TS],
                    )
                    nc.scalar.copy(
                        st["srow"][0:1, 2 * ts:2 * ts + 2, :], pq[:]
                    )

            def norm_chain(b, st):
                """Sliding-window-64 sums + 1/sqrt; result to rdram[b]."""
                srow = st["srow"]
                sqc = slidep.tile([NCH, CPAD], BF16, tag="slide",
                                  name=f"sqc{b}")
                nc.sync.dma_start(sqc[:, 0:CW], srow[0:1, 0:NCH, :])
                nc.sync.dma_start(
                    sqc[:, CW:CPAD], srow[0:1, 1:NCH + 1, 0:64]
                )
                cur = sqc
                width = CPAD
                for sh in (1, 2, 4, 8, 16, 32):
                    width -= sh
                    nxt = slidep.tile([NCH, CPAD], BF16, tag="slide",
                                      name=f"sl{b}_{sh}")
                    nc.vector.tensor_tensor(
                        nxt[:, 0:width], cur[:, 0:width],
                        cur[:, sh:sh + width], op=ALU.add,
                    )
                    cur = nxt
                assert width == CW + 1
                rec = rowp.tile([NCH, CPAD], F32, tag="rec", name=f"rec{b}")
                nc.vector.reciprocal(rec[:, 0:CW], cur[:, 0:CW])
                rc = rcp.tile([NCH, CW], BF16, tag="rc", name=f"rc{b}")
                nc.scalar.activation(rc[:], rec[:, 0:CW], AF.Sqrt)
                nc.sync.dma_start(rdram[b], rc[:])

            def conv_batch(b, st, inter_st):
                """Conv waves + cosine epilogue; sq matmuls of the *next*
                batch (inter_st) are interleaved between waves to keep the
                PE stream dense."""
                xf8 = st["xf8"]
                acc = accp.tile([F, NTILES], F32, name=f"acc{b}")
                sq_chunks = [(0, 3), (3, 6), (6, 8)]
                for wi, w0 in enumerate(range(0, NTILES, WAVE)):
                    wtiles = list(range(w0, min(w0 + WAVE, NTILES)))
                    pcs = {}
                    for ts in wtiles:
                        pcs[ts] = pconv.tile(
                            [F, TS], F32, name=f"pc_{b}_{ts}", tag="pc"
                        )
                    # kq-outer: one stationary serves len(wtiles) matmuls;
                    # followers skip the redundant LDWEIGHTS
                    for kq in range(KQ):
                        for i, ts in enumerate(wtiles):
                            t0 = ts * TS
                            nt = min(TS, TOUT - t0)
                            nt_mm = nt + (nt & 1)
                            bi = nc.tensor.matmul(
                                pcs[ts][:, 0:nt_mm],
                                wdr[:, kq, 0:2, :],
                                xf8[:, 0:2, t0 + 2 * kq: t0 + 2 * kq + nt_mm],
                                start=(kq == 0),
                                stop=(kq == KQ - 1),
                                perf_mode=DR,
                            )
                            if i > 0:
                                bi.ins.ldweights = False
                    if inter_st is not None:
                        lo, hi = sq_chunks[wi]
                        sq_mms(inter_st, range(lo, hi))
                    for ts in wtiles:
                        t0 = ts * TS
                        nt = min(TS, TOUT - t0)
                        # 1/norm row, broadcast across partitions by DMA
                        # from the DRAM copy (not PE)
                        pbs = pbsp.tile(
                            [128, TS], BF16, tag="pbs", name=f"pbs{b}_{ts}"
                        )
                        nc.sync.dma_start(
                            pbs[:],
                            rdram[b, 2 * ts:2 * ts + 2, 0:CW]
                            .partition_broadcast(128),
                        )
                        # DVE may read only one PSUM operand: take |conv| on
                        # ScalarE (PSUM->SBUF), then multiply+row-sum on DVE.
                        scr = scrp.tile(
                            [F, TS], BF16, tag="scr", name=f"scr{b}_{ts}"
                        )
                        nc.scalar.activation(
                            scr[:, 0:nt], pcs[ts][:, 0:nt], AF.Abs
                        )
                        nc.vector.scalar_tensor_tensor(
                            scr[:, 0:nt],
                            scr[:, 0:nt],
                            1.0,
                            pbs[:, 0:nt],
                            op0=ALU.mult,
                            op1=ALU.mult,
                            accum_out=acc[:, ts:ts + 1],
                        )
                nc.vector.reduce_sum(
                    S[:, b:b + 1], acc[:], axis=mybir.AxisListType.X
                )

            # software pipeline: batch b+1 squares/norms and batch b+2
            # loads/casts overlap batch b's conv
            sts = {0: load_batch(0)}
            sq_mms(sts[0], range(NTILES))
            norm_chain(0, sts[0])
            sts[1] = load_batch(1)
            for b in range(BLOC):
                conv_batch(b, sts[b], sts.get(b + 1))
                if b + 2 < BLOC:
                    sts[b + 2] = load_batch(b + 2)
                if b + 1 < BLOC:
                    norm_chain(b + 1, sts[b + 1])

            # ---- finish: out[b] = sum_f g[f]*S[f,b] + sum_f bias[f] ------
            pf = pconv.tile([1, BLOC], F32, name="pf", tag="pc")
            nc.tensor.matmul(pf[:], gsb[:], S[:])
            out_sb = constp.tile([1, BLOC], F32)
            nc.scalar.add(out_sb[:], pf[:], bsumsb[0:1, 0:1])
            nc.sync.dma_start(out_d[:], out_sb[:])

    return nc


_PROGRAM: bass.Bass | None = None


def _get_program() -> bass.Bass:
    global _PROGRAM
    if _PROGRAM is None:
        _PROGRAM = build_program()
    return _PROGRAM


# ---------------------------------------------------------------------------
# Host entry point
# ---------------------------------------------------------------------------

def host_params(conv_weights, spat_weights, weight, bias):
    """Tiny host-side precomputation of stationaries and scalars."""
    conv = np.asarray(conv_weights, dtype=np.float64)
    spat = np.asarray(spat_weights, dtype=np.float64)
    w = np.asarray(weight, dtype=np.float64)
    bb = np.asarray(bias, dtype=np.float64)

    # prod[k, c, f] = conv[f, k] * spat[f, c]; DoubleRow packing
    # k = 32*j + 2*kq + k2  ->  wdr[(k2,c), kq, j, f]
    prod = np.einsum("fk,fc->kcf", conv, spat) * SW    # [K, C, F]
    P5 = prod.reshape(2, KQ, 2, CIN, F)                # [j, kq, k2, c, f]
    wdr = np.ascontiguousarray(P5.transpose(2, 3, 1, 0, 4)).reshape(
        128, KQ, 2, F
    )
    wdr = np.clip(wdr, -240.0, 240.0)
    f8np = mybir.dt.np(F8)
    wdr = wdr.astype(np.float32).astype(f8np)

    norm_w = np.sqrt((spat * spat).sum(1) * (conv * conv).sum(1))  # [F]
    g = (SCALE / (TOUT * norm_w * SW * SX) * w).astype(np.float32).reshape(F, 1)
    bsum = np.array([[bb.sum()]], dtype=np.float32)
    return wdr, g, bsum


def make_in_maps(x, conv_weights, spat_weights, weight, bias):
    x = np.ascontiguousarray(np.asarray(x, dtype=np.float32))
    wdr, g, bsum = host_params(conv_weights, spat_weights, weight, bias)
    bf16np = mybir.dt.np(BF16)
    in_maps = []
    for c in range(NCORES):
        in_maps.append(
            {
                "xs": np.ascontiguousarray(x[c * BLOC:(c + 1) * BLOC]),
                "wdr": wdr,
                "g": g,
                "bsum": bsum,
                "ones64": np.ones((64, 1), bf16np),
            }
        )
    return in_maps


def kernel(x, conv_weights, spat_weights, weight, bias):
    in_maps = make_in_maps(x, conv_weights, spat_weights, weight, bias)
    nc = _get_program()
    res = run_bass_kernel_spmd(nc, in_maps, core_ids=list(range(NCORES)))
    out = np.concatenate(
        [res.results[c]["out"].reshape(BLOC) for c in range(NCORES)]
    )
    return out.astype(np.float32)
